# revision 52
# baseline (speedup 1.0000x reference)
"""nn_Loss_20212116095273 Trainium2 Bass kernel.

out[t,p] = 0.99 * smooth_l1(outputs - targets[...,4:8]).sum(-1)/P
           + 0.01 * 0.2/(P*15) * sum(arccos(cos_angle over 5 bbox points))

Sharding: pedestrian axis split across 8 NeuronCores (25088 padded peds/core,
196 peds per SBUF partition). Direction loss uses the identity
  arccos(cos(v1,v2)) = pi/2 - arctan(dot / |cross|),   1/|cross| = ARS(cross)^2
(ARS = abs_reciprocal_sqrt ACT LUT), so the pi/2 * Npoints constant folds into
the final bias. Per-core partial arctan sums are AllGathered (scalar
collective), then each core applies out = s99*raw_sl1 + (C1 - C2*gsum).

x/y coordinate pairs are processed in single merged instructions via
dim-reordered access patterns; products/dots/crosses use swap and mix views
(negative-step outer dims) over consolidated delta tiles so the whole
bilinear stage runs as 14 wide bf16 DVE/Pool ops per chunk.
"""

import math
import numpy as np

T = 16
F = 15                     # frame pairs in direction loss
P_FULL = 200_000
N_CORES = 8
PS = 25_088                # padded peds per core (128 * 196)
J = PS // 128              # peds per partition per core = 196
JC = 28                    # peds per partition per chunk
NCH = J // JC              # 7 chunks
EPS = 1e-12

S99 = 0.99 / P_FULL
C2 = 0.01 * 0.2 / (P_FULL * F)
C1 = C2 * (P_FULL * F * 5) * (math.pi / 2.0)

_CACHE = {}


def _build(reps: int = 1):
    import concourse.bacc as bacc
    import concourse.mybir as mybir
    import concourse.tile as tile
    import concourse.bass_isa as bass_isa

    f32 = mybir.dt.float32
    bf16 = mybir.dt.bfloat16
    A = mybir.AluOpType
    AF = mybir.ActivationFunctionType

    nc = bacc.Bacc("TRN2", target_bir_lowering=False, debug=False,
                   num_devices=N_CORES)
    tgt = nc.dram_tensor("tgt", [T, PS, 8], f32, kind="ExternalInput")
    outp = nc.dram_tensor("outp", [T, PS, 4], f32, kind="ExternalInput")
    omap = nc.dram_tensor("omap", [T, PS], f32, kind="ExternalOutput")

    tgt_v = tgt.ap().rearrange("t (p j) k -> p t j k", p=128)
    out_v = outp.ap().rearrange("t (p j) k -> p t j k", p=128)
    omap_v = omap.ap().rearrange("t (p j) -> p t j", p=128)

    W = F * JC             # dir stream width per chunk (15*28 = 420)
    W2 = 2 * W
    WS = T * JC * 4        # sl1 width (1792)
    WM = T * JC            # map width (448)

    with tile.TileContext(nc) as tc:
        with (
            tc.tile_pool(name="pin", bufs=2) as pin,
            tc.tile_pool(name="pdel", bufs=2) as pdel,
            tc.tile_pool(name="pscr", bufs=1) as pscr,
            tc.tile_pool(name="ppr", bufs=2) as ppr,
            tc.tile_pool(name="pdc", bufs=1) as pdc,
            tc.tile_pool(name="psc", bufs=2) as psc,
            tc.tile_pool(name="pq", bufs=3) as pq,
            tc.tile_pool(name="pat", bufs=2) as pat,
            tc.tile_pool(name="psd", bufs=NCH) as psd,
            tc.tile_pool(name="psl", bufs=6) as psl,
            tc.tile_pool(name="pmap", bufs=NCH) as pmap,
            tc.tile_pool(name="pmisc", bufs=1) as pmisc,
            tc.tile_pool(name="pdram", bufs=1, space="DRAM") as pdram,
        ):
          for _rep in range(reps):
            acc = pmisc.tile([128, NCH], f32, tag="acc")
            epsb = pmisc.tile([128, 1], f32, tag="epsb")
            nc.vector.memset(epsb[:], EPS)
            neg1 = pmisc.tile([128, 1], f32, tag="neg1")
            nc.vector.memset(neg1[:], -1.0)
            raw_maps = []
            q_tiles = []
            sd_tiles = []

            def emit_sl1(ci):
                # sl1(d) = 0.5*min(d^2,1) + relu(d-1) + relu(-d-1)
                sd = sd_tiles[ci]
                g1 = psl.tile([128, WS], bf16, tag="sl1")
                g2 = psl.tile([128, WS], bf16, tag="sl1")
                sq = psl.tile([128, WS], bf16, tag="sl1")
                msq = psl.tile([128, WS], bf16, tag="sl1")
                s1 = psl.tile([128, WS], bf16, tag="sl1")
                s2 = psl.tile([128, WS], bf16, tag="sl1")
                nc.scalar.activation(g1[:], sd[:], AF.Relu, bias=neg1[:])
                nc.scalar.activation(g2[:], sd[:], AF.Relu, bias=neg1[:],
                                     scale=-1.0)
                nc.scalar.activation(sq[:], sd[:], AF.Square)
                nc.vector.tensor_scalar(msq[:], sq[:], 1.0, None, A.min)
                nc.vector.scalar_tensor_tensor(s1[:], msq[:], 0.5, g1[:],
                                               A.mult, A.add)
                nc.vector.tensor_add(s2[:], s1[:], g2[:])
                s2v = s2[:].rearrange("p (t j k) -> p t j k", t=T, j=JC, k=4)
                r1 = psc.tile([128, T * JC * 2], bf16, tag="r1")
                r1v = r1[:].rearrange("p (t j k) -> p t j k", t=T, j=JC, k=2)
                nc.vector.tensor_add(r1v, s2v[:, :, :, 0:2], s2v[:, :, :, 2:4])
                raw = pmap.tile([128, WM], bf16, tag="rawmap")
                rawv = raw[:].rearrange("p (t j) -> p t j", t=T)
                nc.vector.tensor_add(rawv, r1v[:, :, :, 0:1], r1v[:, :, :, 1:2])
                raw_maps.append(raw)

            def emit_arctan(lo, hi):
                for cj in range(lo, hi):
                    at = pat.tile([128, 5 * W], bf16, tag="at")
                    nc.scalar.activation(at[:], q_tiles[cj][:], AF.Arctan,
                                         accum_out=acc[:, cj:cj + 1])

            for ci in range(NCH):
                j0 = ci * JC
                tt = pin.tile([128, T * JC * 8], f32, tag="tt")
                ot = pin.tile([128, T * JC * 4], f32, tag="ot")
                nc.sync.dma_start(
                    tt[:].rearrange("p (t j k) -> p t j k", t=T, j=JC, k=8),
                    tgt_v[:, :, j0:j0 + JC, :])
                nc.sync.dma_start(
                    ot[:].rearrange("p (t j k) -> p t j k", t=T, j=JC, k=4),
                    out_v[:, :, j0:j0 + JC, :])
                t4 = tt[:].rearrange("p (t j k) -> p t j k", t=T, j=JC, k=8)
                o4 = ot[:].rearrange("p (t j k) -> p t j k", t=T, j=JC, k=4)

                # k-merged (x,y) views: dims ordered [k, rows, j]
                def tk(r0, r1, k0):
                    return t4[:, r0:r1, :, k0:k0 + 2].rearrange(
                        "p r j k -> p k r j")

                def okk(r0, r1, k0):
                    return o4[:, r0:r1, :, k0:k0 + 2].rearrange(
                        "p r j k -> p k r j")

                ab, ab1 = tk(0, F, 0), tk(1, F + 1, 0)       # (a,b) frames
                cd, cd1 = tk(0, F, 2), tk(1, F + 1, 2)       # (c,d) frames
                oab, ocd = okk(0, F, 0), okk(0, F, 2)
                ab0r, cd0r = tk(0, 1, 0), tk(0, 1, 2)
                oab0, ocd0 = okk(0, 1, 0), okk(0, 1, 2)

                # delta tiles: PD = [pX0|pY0|pX1|pY1|pcx|pcy], TD likewise
                PD = pdel.tile([128, 6 * W], bf16, tag="PD")
                TD = pdel.tile([128, 6 * W], bf16, tag="TD")
                PD6 = PD[:].rearrange("p (s n) -> p s n", s=6)
                TD6 = TD[:].rearrange("p (s n) -> p s n", s=6)

                def seg(t6, s0, s1):           # [s1-s0 segments] as k-r-j view
                    return t6[:, s0:s1, :].rearrange(
                        "p k (r j) -> p k r j", r=F)

                u = pscr.tile([128, W2], f32, tag="u")
                v = pscr.tile([128, W2], f32, tag="v")
                w = pscr.tile([128, W2], f32, tag="w")
                z = pscr.tile([128, W2], f32, tag="z")

                def krj(tl):
                    return tl[:].rearrange("p (k r j) -> p k r j", k=2, r=F)

                def krj0(tl):                  # row 0 only
                    return krj(tl)[:, :, 0:1, :]

                uv, vv, wv, zv = krj(u), krj(v), krj(w), krj(z)

                # ---- stage A (merged x/y). pair rows 1..14 = frames 1..14;
                # row 0 (pair 0) fixed up afterwards.
                nc.vector.scalar_tensor_tensor(uv, ab, 0.5, oab, A.mult, A.add)
                nc.vector.scalar_tensor_tensor(seg(PD6, 0, 2), ocd, -0.5, uv,
                                               A.mult, A.add)
                nc.vector.scalar_tensor_tensor(
                    seg(PD6, 0, 2)[:, :, 0:1, :], ocd0, -0.5, oab0,
                    A.mult, A.add)
                nc.vector.scalar_tensor_tensor(seg(PD6, 2, 4), cd, 0.5, oab,
                                               A.mult, A.add)
                nc.vector.tensor_copy(seg(PD6, 2, 4)[:, :, 0:1, :], oab0)
                nc.gpsimd.tensor_sub(vv, ab1, ab)
                nc.vector.scalar_tensor_tensor(seg(TD6, 2, 4), cd, 0.5, vv,
                                               A.mult, A.add)
                nc.vector.tensor_copy(seg(TD6, 2, 4)[:, :, 0:1, :], krj0(v))
                nc.vector.scalar_tensor_tensor(wv, cd1, -0.5, ab1,
                                               A.mult, A.add)
                nc.gpsimd.tensor_sub(zv, ab, cd)
                nc.vector.scalar_tensor_tensor(seg(TD6, 0, 2), zv, -0.5, wv,
                                               A.mult, A.add)
                nc.vector.scalar_tensor_tensor(krj0(z), cd0r, -0.5, ab0r,
                                               A.mult, A.add)
                nc.vector.tensor_sub(seg(TD6, 0, 2)[:, :, 0:1, :], krj0(w),
                                     krj0(z))
                # centers
                nc.gpsimd.tensor_add(PD6[:, 4:6, :], PD6[:, 0:2, :],
                                     PD6[:, 2:4, :])
                nc.gpsimd.tensor_add(TD6[:, 4:6, :], TD6[:, 0:2, :],
                                     TD6[:, 2:4, :])

                # ---- stage B: products / dots / crosses (wide bf16 ops)
                # PR1 = [XX0|YY0|XX1|YY1|Cxx|Cyy]
                PR1 = ppr.tile([128, 6 * W], bf16, tag="PR1")
                PR16 = PR1[:].rearrange("p (s n) -> p s n", s=6)
                # PR2 = [XY00|YX00|XY11|YX11|Cxy|Cyx]
                PR2 = ppr.tile([128, 6 * W], bf16, tag="PR2")
                PR26 = PR2[:].rearrange("p (s n) -> p s n", s=6)
                # PR3 = [XY01|YX10|XY10|YX01]
                PR3 = ppr.tile([128, 4 * W], bf16, tag="PR3")
                PR34 = PR3[:].rearrange("p (s n) -> p s n", s=4)

                nc.vector.tensor_mul(PR16[:, 0:2, :], PD6[:, 0:2, :],
                                     TD6[:, 0:2, :])
                nc.vector.tensor_mul(PR16[:, 2:4, :], PD6[:, 2:4, :],
                                     TD6[:, 2:4, :])
                nc.gpsimd.tensor_mul(PR16[:, 4:6, :], PD6[:, 4:6, :],
                                     TD6[:, 4:6, :])
                nc.vector.tensor_mul(PR26[:, 0:2, :], PD6[:, 0:2, :],
                                     TD6[:, 1::-1, :])
                nc.vector.tensor_mul(PR26[:, 2:4, :], PD6[:, 2:4, :],
                                     TD6[:, 3:1:-1, :])
                nc.vector.tensor_mul(PR26[:, 4:6, :], PD6[:, 4:6, :],
                                     TD6[:, 5:3:-1, :])
                nc.vector.tensor_mul(PR34[:, 0:2, :], PD6[:, 0:4:3, :],
                                     TD6[:, 3::-3, :])
                nc.vector.tensor_mul(PR34[:, 2:4, :], PD6[:, 2:0:-1, :],
                                     TD6[:, 1:3, :])

                # DT = [d00|d11|d01|d10|dcc], CR = [c00|c11|c01|c10|ccc]
                DT = pdc.tile([128, 5 * W], bf16, tag="DT")
                DT5 = DT[:].rearrange("p (s n) -> p s n", s=5)
                CR = pdc.tile([128, 5 * W], bf16, tag="CR")
                CR5 = CR[:].rearrange("p (s n) -> p s n", s=5)
                nc.vector.tensor_add(DT5[:, 0:2, :], PR16[:, 0:3:2, :],
                                     PR16[:, 1:4:2, :])
                nc.vector.tensor_add(DT5[:, 2:4, :], PR16[:, 0:3:2, :],
                                     PR16[:, 3:0:-2, :])
                nc.gpsimd.tensor_add(DT5[:, 4:5, :], PR16[:, 4:5, :],
                                     PR16[:, 5:6, :])
                nc.vector.tensor_sub(CR5[:, 0:2, :], PR26[:, 0:3:2, :],
                                     PR26[:, 1:4:2, :])
                nc.vector.tensor_sub(CR5[:, 2:4, :], PR34[:, 0:3:2, :],
                                     PR34[:, 1:4:2, :])
                nc.gpsimd.tensor_sub(CR5[:, 4:5, :], PR26[:, 4:5, :],
                                     PR26[:, 5:6, :])

                # ---- stage C: q = dot * ARS(cross)^2 ; arctan accumulated
                uu = psc.tile([128, 5 * W], bf16, tag="uu")
                iv = psc.tile([128, 5 * W], bf16, tag="iv")
                q = pq.tile([128, 5 * W], bf16, tag="q")
                nc.scalar.activation(uu[:], CR[:], AF.Abs_reciprocal_sqrt,
                                     bias=epsb[:])
                nc.scalar.activation(iv[:], uu[:], AF.Square)
                nc.vector.tensor_mul(q[:], DT[:], iv[:])
                q_tiles.append(q)

                # ---- smooth L1 d = outputs - delta. For the first chunks
                # the full sl1 chain runs inline (absorbed by main-loop
                # engine gaps); the last two chunks' chains are deferred to
                # overlap the collective latency.
                sd = psd.tile([128, WS], bf16, tag="sd")
                nc.gpsimd.tensor_sub(sd[:], o4, t4[:, :, :, 4:8])
                sd_tiles.append(sd)

            emit_arctan(0, NCH)

            # ---- global reduction of arctan partials
            accs = pmisc.tile([128, 1], f32, tag="accs")
            nc.vector.tensor_reduce(accs[:], acc[:], mybir.AxisListType.X, A.add)
            par = pmisc.tile([128, 1], f32, tag="par")
            nc.gpsimd.partition_all_reduce(par[:], accs[:], 128,
                                           bass_isa.ReduceOp.add)
            cin = pdram.tile([128, 1], f32, tag="cin")
            cout = pdram.tile([N_CORES * 128, 1], f32, tag="cout")
            nc.sync.dma_start(cin[:], par[:])
            nc.gpsimd.collective_compute(
                "AllGather", A.bypass,
                replica_groups=[list(range(N_CORES))],
                ins=[cin.opt()], outs=[cout.opt()])
            # ---- deferred smooth L1 for the last chunks (overlaps the
            # collective)
            for ci in range(NCH):
                emit_sl1(ci)

            # post-collective plumbing + bias emitted after all deferred sl1
            # ops so they do not block the DVE/ACT in-order streams while
            # waiting on the collective
            g8 = pmisc.tile([128, N_CORES], f32, tag="g8")
            nc.sync.dma_start(
                g8[:], cout[:].rearrange("(c p) one -> p (c one)", c=N_CORES))
            gsum = pmisc.tile([128, 1], f32, tag="gsum")
            nc.vector.tensor_reduce(gsum[:], g8[:], mybir.AxisListType.X, A.add)
            c1t = pmisc.tile([128, 1], f32, tag="c1t")
            nc.vector.memset(c1t[:], C1)
            bias = pmisc.tile([128, 1], f32, tag="bias")
            nc.scalar.activation(bias[:], gsum[:], AF.Identity,
                                 bias=c1t[:], scale=-C2)
            for ci in range(NCH):
                j0 = ci * JC
                fin = pat.tile([128, WM], f32, tag="fin")
                nc.scalar.activation(fin[:], raw_maps[ci][:], AF.Identity,
                                     bias=bias[:], scale=S99)
                nc.sync.dma_start(
                    omap_v[:, :, j0:j0 + JC],
                    fin[:].rearrange("p (t j) -> p t j", t=T))

    nc.compile()
    return nc


def get_program():
    if "nc" not in _CACHE:
        _CACHE["nc"] = _build()
    return _CACHE["nc"]


def kernel(outputs: np.ndarray, targets: np.ndarray) -> np.ndarray:
    from concourse import bass_utils

    nc = get_program()
    outputs = np.ascontiguousarray(outputs, dtype=np.float32)
    targets = np.ascontiguousarray(targets, dtype=np.float32)

    in_maps = []
    for c in range(N_CORES):
        lo = c * 25000
        t_s = np.zeros((T, PS, 8), np.float32)
        o_s = np.zeros((T, PS, 4), np.float32)
        t_s[:, :25000] = targets[:, lo:lo + 25000]
        o_s[:, :25000] = outputs[:, lo:lo + 25000]
        in_maps.append({"tgt": t_s, "outp": o_s})

    res = bass_utils.run_bass_kernel_spmd(nc, in_maps,
                                          core_ids=list(range(N_CORES)))
    out = np.empty((T, P_FULL), np.float32)
    for c in range(N_CORES):
        out[:, c * 25000:(c + 1) * 25000] = res.results[c]["omap"][:, :25000]
    return out
